# revision 1
# baseline (speedup 1.0000x reference)
"""Chunked causal self-attention with RoPE on 8 Trainium2 NeuronCores.

Problem: B=4, L=4096, H=16, DH=DV=128, CHUNK=1024 (N=4 chunks).
RoPE on q,k then chunk-local causal attention per (batch, chunk, head).

Sharding: heads split across 8 cores (2 heads/core) -> 32 independent
(1024 x 1024, d=128) attention problems per core, grouped 4-per-(b,h)
so one RoPE table pass covers a whole group.

v2 design notes (from trace analysis of v1):
  - All inputs packed d-major as (128, NPROB*1024) fp16 so every group
    load is one DMA with 128 x 8KB descriptors; a single sync-queue
    stream sustains ~400GB/s this way.
  - PE work: per problem 12 score matmuls (4608 cols), 12 PV matmuls
    (4608 cols), 2 denominator matmuls over an fp16 block-sum R (1024
    cols). No mask matmuls: causal diag masking is a DVE multiply with
    an upper-tri 0/1 tile on the fp16 P-tile.
  - Software pipeline one problem deep (PE order: PV(p-1), den(p-1),
    scores(p)) keeps the PE continuously busy so it reaches the 2.4GHz
    p-state instead of 1.2GHz.
  - exp on ScalarE; narrow blocks are paired into shared PSUM tiles so
    there are 6 activation instructions per problem instead of 8.
  - Normalization (outT/den) and final layout transposes on host.
"""

import math
import os
import sys

import numpy as np

for _p in ("/opt/trn_rl_repo", "/root/.axon_site/_ro/trn_rl_repo"):
    if os.path.isdir(_p) and _p not in sys.path:
        sys.path.insert(0, _p)

import concourse.bass as bass  # noqa: E402
import concourse.tile as tile  # noqa: E402
from concourse import bass_utils, mybir  # noqa: E402

B, L, H, DH, DV = 4, 4096, 16, 128, 128
CHUNK = 1024
NCHUNK = L // CHUNK  # 4
ROPE_BASE = 10000.0
NCORES = 8
HPC = H // NCORES  # 2 heads per core
NPROB = B * HPC * NCHUNK  # 32 problems per core
NG = B * HPC  # 8 groups of 4 chunks
HALF = DH // 2  # 64
NB = CHUNK // 128  # 8 k-blocks
SCALE = 1.0 / math.sqrt(DH)

F16 = mybir.dt.float16
F32 = mybir.dt.float32
AF = mybir.ActivationFunctionType

# exp "units": list of (psS column offset, block) pairs per unit.
# Blocks 0-3 get their own [128,1024] PSUM tile; (4,5) and (6,7) share.
# Each entry: (unit_tag, [(block, tile_col_off)])
UNITS = [
    ("u0", [(0, 0)]),
    ("u1", [(1, 128)]),
    ("u2", [(2, 256)]),
    ("u3", [(3, 384)]),
    ("u45", [(4, 0), (5, 512)]),
    ("u67", [(6, 0), (7, 256)]),
]
# For single blocks the tile col offset equals q0 so piece splits at 512
# stay bank-aligned. For paired units each block's region starts at a
# bank boundary (0 / 512) or stays within one bank.


def _block_region(b):
    """absolute q range covered for k-block b (causal)."""
    return 128 * b, CHUNK


def build_module(nprob=NPROB):
    from concourse import bacc

    nc = bacc.Bacc("TRN2", target_bir_lowering=False, debug=False)

    qT = nc.dram_tensor("qT_in", (128, nprob * CHUNK), F16, kind="ExternalInput")
    kT = nc.dram_tensor("kT_in", (128, nprob * CHUNK), F16, kind="ExternalInput")
    vT = nc.dram_tensor("vT_in", (128, nprob * CHUNK), F16, kind="ExternalInput")
    tri = nc.dram_tensor("tri_in", (128, 128), F16, kind="ExternalInput")
    ones = nc.dram_tensor("ones_in", (128, 1), F16, kind="ExternalInput")

    outT = nc.dram_tensor("outT_out", (128, nprob * CHUNK), F16, kind="ExternalOutput")
    den = nc.dram_tensor(
        "den_out", (nprob // NCHUNK, NCHUNK * CHUNK), F16, kind="ExternalOutput"
    )

    with tile.TileContext(nc) as tc:
        _body(tc, nprob, qT, kT, vT, tri, ones, outT, den)
    nc.compile()
    return nc


def _body(tc, nprob, qT, kT, vT, tri, ones, outT, den):
    from contextlib import ExitStack

    nc = tc.nc
    ngroups = nprob // NCHUNK
    GW = NCHUNK * CHUNK  # group width: 4096 cols

    with ExitStack() as ctx:
        consts = ctx.enter_context(tc.tile_pool(name="consts", bufs=1))
        ing = ctx.enter_context(tc.tile_pool(name="ing", bufs=2))
        qkp = ctx.enter_context(tc.tile_pool(name="qkp", bufs=2))
        ptp = ctx.enter_context(tc.tile_pool(name="ptp", bufs=16))
        rp = ctx.enter_context(tc.tile_pool(name="rp", bufs=2))
        outp = ctx.enter_context(tc.tile_pool(name="outp", bufs=2))
        dnp = ctx.enter_context(tc.tile_pool(name="dnp", bufs=2))
        psSp = ctx.enter_context(tc.tile_pool(name="psS", bufs=3, space="PSUM"))
        psOp = ctx.enter_context(tc.tile_pool(name="psO", bufs=1, space="PSUM"))

        tri_t = consts.tile([128, 128], F16, tag="tri")
        nc.sync.dma_start(out=tri_t, in_=tri.ap())
        ones_t = consts.tile([128, 1], F16, tag="ones")
        nc.sync.dma_start(out=ones_t, in_=ones.ap())

        # touch consts once so compute ops don't carry extra DMA waits
        dummy = consts.tile([128, 1], F16, tag="dummy")
        nc.vector.tensor_copy(out=dummy, in_=tri_t[:, 0:1])
        nc.vector.tensor_copy(out=dummy, in_=ones_t)

        state = {}  # per live problem: pt tiles, R, group tiles

        def emit_loads_rope(g):
            qg = ing.tile([128, GW], F16, tag="qg")
            nc.sync.dma_start(out=qg, in_=qT.ap()[:, g * GW:(g + 1) * GW])
            kg = ing.tile([128, GW], F16, tag="kg")
            nc.sync.dma_start(out=kg, in_=kT.ap()[:, g * GW:(g + 1) * GW])
            vg = ing.tile([128, GW], F16, tag="vg")
            nc.sync.dma_start(out=vg, in_=vT.ap()[:, g * GW:(g + 1) * GW])
            state[("grp", g)] = (vg, qg, kg)

        def emit_scores_exp(p):
            g, pi = divmod(p, NCHUNK)
            vg, qp, kp = state[("grp", g)]
            poff = pi * CHUNK
            pts = {}
            for tag, blocks in UNITS:
                ps = psSp.tile([128, CHUNK], F32, tag="psS")
                # score matmuls for each block in this unit
                for b, off in blocks:
                    q0, q1 = _block_region(b)
                    kblk = kp[:, poff + 128 * b: poff + 128 * (b + 1)]
                    # pieces of [q0,q1) split at bank boundaries rel. tile
                    # tile col of abs q is (q - q0 + off)
                    a = q0
                    while a < q1:
                        # bank boundary in tile coords
                        ta = a - q0 + off
                        bank_end = ((ta // 512) + 1) * 512
                        e = min(q1, a + (bank_end - ta))
                        nc.tensor.matmul(
                            ps[:, ta:ta + (e - a)],
                            lhsT=kblk,
                            rhs=qp[:, poff + a: poff + e],
                            start=True, stop=True,
                        )
                        a = e
                pt = ptp.tile([128, CHUNK], F16, tag="pt")
                # one exp over the full used span of this unit tile
                lo_off = min(off for b, off in blocks)
                hi_off = max(off + (CHUNK - 128 * b) for b, off in blocks)
                nc.scalar.activation(
                    out=pt[:, lo_off:hi_off], in_=ps[:, lo_off:hi_off],
                    func=AF.Exp, scale=SCALE,
                )
                for b, off in blocks:
                    pts[b] = (pt, off)
            state[("pt", p)] = pts

        def emit_masks_r(p):
            pts = state[("pt", p)]
            # diag masks in place: blocks 0-3 on DVE, 4-7 on GpSimd
            for b in range(NB):
                ptb, offb = pts[b]
                eng = nc.vector if b < 4 else nc.gpsimd
                eng.tensor_mul(
                    ptb[:, offb:offb + 128], ptb[:, offb:offb + 128], tri_t
                )
            # R = sum over all blocks (DVE)
            R = rp.tile([128, CHUNK], F16, name="R", tag="R")
            pt0, off0 = pts[0]
            nc.vector.tensor_copy(out=R, in_=pt0[:, off0:off0 + CHUNK])
            for b in range(1, NB):
                ptb, offb = pts[b]
                q0, q1 = _block_region(b)
                nc.vector.tensor_add(
                    R[:, q0:q1], R[:, q0:q1], ptb[:, offb:offb + (q1 - q0)]
                )
            state[("R", p)] = R

        def emit_pv_den(p):
            g, pi = divmod(p, NCHUNK)
            vg, qp, kp = state[("grp", g)]
            poff = pi * CHUNK
            pts = state.pop(("pt", p))
            R = state.pop(("R", p))
            pso = psOp.tile([128, CHUNK], F32, tag="psO")
            # accumulate over blocks; per psum bank the last writer stops
            last_in_bank = {0: 3, 1: NB - 1}  # bank0 cols [0,512): blocks 0..3
            for b in range(NB):
                q0, q1 = _block_region(b)
                ptb, offb = pts[b]
                vblk = vg[:, poff + 128 * b: poff + 128 * (b + 1)]
                a = q0
                while a < q1:
                    bank = a // 512
                    e = min(q1, (bank + 1) * 512)
                    nc.tensor.matmul(
                        pso[:, a:e],
                        lhsT=vblk,
                        rhs=ptb[:, offb + (a - q0): offb + (e - q0)],
                        start=(b == 0),
                        stop=(b == last_in_bank[bank]),
                    )
                    a = e
            psd = psSp.tile([128, CHUNK], F32, name="psd", tag="psS")
            nc.tensor.matmul(psd[0:1, 0:512], lhsT=ones_t, rhs=R[:, 0:512],
                             start=True, stop=True)
            nc.tensor.matmul(psd[0:1, 512:CHUNK], lhsT=ones_t, rhs=R[:, 512:CHUNK],
                             start=True, stop=True)

            # drain psO (GpSimd) / psd (DVE) to SBUF fp16, batched per group
            if pi == 0:
                state[("outg", g)] = outp.tile([128, GW], F16, name="outg", tag="outg")
                state[("deng", g)] = dnp.tile([1, GW], F16, name="deng", tag="deng")
            outg = state[("outg", g)]
            deng = state[("deng", g)]
            nc.vector.tensor_copy(out=outg[:, poff:poff + CHUNK], in_=pso)
            nc.vector.tensor_copy(out=deng[:, poff:poff + CHUNK], in_=psd[0:1, :])
            if pi == NCHUNK - 1:
                nc.sync.dma_start(
                    out=outT.ap()[:, g * GW:(g + 1) * GW], in_=outg
                )
                nc.sync.dma_start(out=den.ap()[g], in_=deng)
                state.pop(("outg", g))
                state.pop(("deng", g))
                state.pop(("grp", g))

        # main software-pipelined loop; per step: scores(p) first on PE so
        # it never waits on exp(p-1), then PV/den(p-1).
        for p in range(nprob + 1):
            if p < nprob:
                if p == 0:
                    emit_loads_rope(0)
                if p % NCHUNK == 1 and (p // NCHUNK) + 1 < ngroups:
                    emit_loads_rope(p // NCHUNK + 1)
                emit_scores_exp(p)
            if p > 0:
                emit_pv_den(p - 1)
            if p < nprob:
                emit_masks_r(p)


def _host_consts():
    freqs = np.exp(np.arange(HALF, dtype=np.float64) * (-math.log(ROPE_BASE) / HALF))
    pos = np.arange(L, dtype=np.float64)
    ang = pos[:, None] * freqs[None, :]  # (L, 64)
    cos = np.cos(ang)
    sin = np.sin(ang)
    r = np.arange(128)
    tri = (r[None, :] >= r[:, None]).astype(np.float16)  # keep q >= k
    ones = np.ones((128, 1), np.float16)
    return cos, sin, tri, ones


def _rope_host(x, cos, sin):
    """x: (B, L, HPC, 128) fp32; cos/sin: (L, 64)."""
    x1, x2 = x[..., :HALF], x[..., HALF:]
    c = cos[None, :, None, :]
    s = sin[None, :, None, :]
    return np.concatenate([x1 * c - x2 * s, x2 * c + x1 * s], axis=-1)


def _pack_core(qc, kc, vc, cos, sin):
    """qc,kc,vc: (B, L, HPC, 128) fp32 -> d-major (128, NPROB*1024) fp16."""
    qc = _rope_host(qc, cos, sin)
    kc = _rope_host(kc, cos, sin)

    def dmaj(x):
        # (B, L, h, D) -> (b, h, n, j, d) -> (d, b, h, n, j)
        a = x.transpose(0, 2, 1, 3).reshape(B, HPC, NCHUNK, CHUNK, DH)
        a = a.transpose(4, 0, 1, 2, 3).reshape(DH, NPROB * CHUNK)
        return np.ascontiguousarray(a).astype(np.float16)

    # v: partition = k-within-block, cols = (b,h,n, block, dv)
    a = vc.transpose(0, 2, 1, 3).reshape(B, HPC, NCHUNK, NB, 128, DV)
    a = a.transpose(4, 0, 1, 2, 3, 5).reshape(128, NPROB * CHUNK)
    vp = np.ascontiguousarray(a).astype(np.float16)
    return dict(qT_in=dmaj(qc), kT_in=dmaj(kc), vT_in=vp)


_NC_CACHE = {}
LAST_RESULT = None


def _get_module(nprob=NPROB):
    if nprob not in _NC_CACHE:
        _NC_CACHE[nprob] = build_module(nprob)
    return _NC_CACHE[nprob]


def kernel(q, k, v):
    q = np.asarray(q, dtype=np.float32)
    k = np.asarray(k, dtype=np.float32)
    v = np.asarray(v, dtype=np.float32)

    cos, sin, tri, ones = _host_consts()
    consts = dict(tri_in=tri, ones_in=ones)

    in_maps = []
    for c in range(NCORES):
        hs = slice(HPC * c, HPC * (c + 1))
        m = _pack_core(q[:, :, hs], k[:, :, hs], v[:, :, hs], cos, sin)
        m.update(consts)
        in_maps.append(m)

    nc = _get_module(NPROB)
    trace = bool(int(os.environ.get("KERNEL_TRACE", "0")))
    res = bass_utils.run_bass_kernel_spmd(
        nc, in_maps, core_ids=list(range(NCORES)), trace=trace
    )
    global LAST_RESULT
    LAST_RESULT = res

    out = np.empty((B, L, H, DV), np.float32)
    for c in range(NCORES):
        ot = res.results[c]["outT_out"].astype(np.float32)  # (128dv, 32*1024)
        dn = res.results[c]["den_out"].astype(np.float32).reshape(-1)  # (32*1024,)
        o = ot / dn[None, :]
        # (dv, b, h, n, j) -> (b, n*j=L, h, dv)
        o = o.reshape(DV, B, HPC, NCHUNK, CHUNK).transpose(1, 3, 4, 2, 0)
        out[:, :, HPC * c:HPC * (c + 1)] = o.reshape(B, L, HPC, DV)
    return out



# revision 11
# speedup vs baseline: 1.2106x; 1.2106x over previous
"""Chunked causal self-attention with RoPE on 8 Trainium2 NeuronCores.

Problem: B=4, L=4096, H=16, DH=DV=128, CHUNK=1024 (N=4 chunks).
RoPE on q,k then chunk-local causal attention per (batch, chunk, head).

Sharding: heads split across 8 cores (2 heads/core) -> 32 independent
(1024 x 1024, d=128) attention problems per core, grouped 4-per-(b,h)
so one load covers a whole group.

v3 design notes (from trace analysis of v2: DVE was 89.5% busy on the
softmax-denominator R-build + fp32 PSUM drains; ACT 69%, PE 72%):
  - Fused denominator: PV runs transposed, out[q,dv] = sum_k P[k,q] *
    Vext[k,dv] with Vext = [V | ones]; the 129th output column IS the
    softmax denominator. This deletes the DVE R-build, the PE den
    matmuls and the separate den drain.
  - exp in 4 activations/problem (block units {b0,b4} {b1,b7} {b2,b3}
    {b5,b6} = 1536/1024/1408/640 cols) to amortize ScalarE overhead.
  - PSUM: psA [128,1536] (3 banks) + psB [128,1024] (2) ping-pong for
    scores; psO [128,1536] (3) holds the 8 q-block PV outputs at
    offsets {0,129,258, 512,641,770, 1024,1153} so no matmul output
    crosses a bank. Total exactly 8 banks.
  - DVE only does 8 diag masks + 3 compacting PSUM->SBUF casts per
    problem (~3us vs ~7us in v2).
  - Normalization (num/den) and final layout transposes on host.
"""

import math
import os
import sys

import numpy as np

for _p in ("/opt/trn_rl_repo", "/root/.axon_site/_ro/trn_rl_repo"):
    if os.path.isdir(_p) and _p not in sys.path:
        sys.path.insert(0, _p)

import concourse.bass as bass  # noqa: E402
import concourse.tile as tile  # noqa: E402
from concourse import bass_utils, mybir  # noqa: E402

B, L, H, DH, DV = 4, 4096, 16, 128, 128
CHUNK = 1024
NCHUNK = L // CHUNK  # 4
ROPE_BASE = 10000.0
NCORES = 8
HPC = H // NCORES  # 2 heads per core
NPROB = B * HPC * NCHUNK  # 32 problems per core
NG = B * HPC  # 8 groups of 4 chunks
HALF = DH // 2  # 64
NB = CHUNK // 128  # 8 k-blocks
SCALE = 1.0 / math.sqrt(DH)
VW = 130  # v block width fed to PV: 128 dv + ones col (den) + zero pad
VS = 132  # v block stride in SBUF/HBM (8B-aligned: 132*2B = 264B)
PW = NB * VS  # 1056 packed v cols per problem
OW = 130  # psO q-block region stride (130*4B = 520B, 8B-aligned)
POW = NB * OW  # 1040 packed output cols per problem

F16 = mybir.dt.float16
F32 = mybir.dt.float32
AF = mybir.ActivationFunctionType

# exp units: (tag, psum pool key, [(block, tile_col_off)], span)
# unit tile sizes: A=1536 (3 banks), B=1024 (2 banks); A and B ping-pong.
UNITS = [
    ("uA", "A", [(0, 0), (4, 1024)], 1536),
    ("uB", "B", [(1, 0), (7, 896)], 1024),
    ("uC", "A", [(2, 0), (3, 768)], 1408),
    ("uD", "B", [(5, 0), (6, 384)], 640),
]
# block -> (unit index, tile col offset of block start)
BLK = {}
for _ui, (_, _, _blocks, _) in enumerate(UNITS):
    for _b, _off in _blocks:
        BLK[_b] = (_ui, _off)

# psO column offset of each q-block's 129-wide output region (bank safe:
# banks hold 512 fp32; 3+3+2 regions per bank; 8B-aligned starts).
QB_OFF = [0, 130, 260, 512, 642, 772, 1024, 1154]
# drain: (psO src range) -> (outg dst offset); keeps the 130-col stride
DRAINS = [(0, 390, 0), (512, 902, 390), (1024, 1284, 780)]


def _block_region(b):
    """absolute q range covered for k-block b (causal)."""
    return 128 * b, CHUNK


def build_module(nprob=NPROB):
    from concourse import bacc

    nc = bacc.Bacc("TRN2", target_bir_lowering=False, debug=False)

    qT = nc.dram_tensor("qT_in", (128, nprob * CHUNK), F16, kind="ExternalInput")
    kT = nc.dram_tensor("kT_in", (128, nprob * CHUNK), F16, kind="ExternalInput")
    vT = nc.dram_tensor("vT_in", (128, nprob * PW), F16, kind="ExternalInput")
    tri = nc.dram_tensor("tri_in", (128, 128), F16, kind="ExternalInput")

    outT = nc.dram_tensor("outT_out", (128, nprob * POW), F16, kind="ExternalOutput")

    with tile.TileContext(nc) as tc:
        _body(tc, nprob, qT, kT, vT, tri, outT)
    nc.compile()
    return nc


def _body(tc, nprob, qT, kT, vT, tri, outT):
    from contextlib import ExitStack

    nc = tc.nc
    ngroups = nprob // NCHUNK
    GW = NCHUNK * CHUNK  # q/k group width: 4096 cols
    GV = NCHUNK * PW  # v group width: 4224 cols
    GO = NCHUNK * POW  # out group width: 4160 cols

    with ExitStack() as ctx:
        consts = ctx.enter_context(tc.tile_pool(name="consts", bufs=1))
        ing = ctx.enter_context(tc.tile_pool(name="ing", bufs=2))
        ptp = ctx.enter_context(tc.tile_pool(name="ptp", bufs=2))
        outp = ctx.enter_context(tc.tile_pool(name="outp", bufs=2))
        psAp = ctx.enter_context(tc.tile_pool(name="psA", bufs=1, space="PSUM"))
        psBp = ctx.enter_context(tc.tile_pool(name="psB", bufs=1, space="PSUM"))
        psOp = ctx.enter_context(tc.tile_pool(name="psO", bufs=1, space="PSUM"))

        tri_t = consts.tile([128, 128], F16, tag="tri")
        nc.sync.dma_start(out=tri_t, in_=tri.ap())

        # touch consts once so compute ops don't carry extra DMA waits
        dummy = consts.tile([128, 1], F16, tag="dummy")
        nc.vector.tensor_copy(out=dummy, in_=tri_t[:, 0:1])

        state = {}

        def emit_loads(g):
            qg = ing.tile([128, GW], F16, tag="qg")
            nc.sync.dma_start(out=qg, in_=qT.ap()[:, g * GW:(g + 1) * GW])
            kg = ing.tile([128, GW], F16, tag="kg")
            nc.sync.dma_start(out=kg, in_=kT.ap()[:, g * GW:(g + 1) * GW])
            vg = ing.tile([128, GV], F16, tag="vg")
            nc.sync.dma_start(out=vg, in_=vT.ap()[:, g * GV:(g + 1) * GV])
            state[("grp", g)] = (qg, kg, vg)

        def emit_scores_exp_unit(p, ui):
            """score matmuls + one exp for unit ui of problem p."""
            g, pi = divmod(p, NCHUNK)
            qg, kg, vg = state[("grp", g)]
            poff = pi * CHUNK
            tag, pool_key, blocks, span = UNITS[ui]
            pool = psAp if pool_key == "A" else psBp
            ps = pool.tile([128, 1536 if pool_key == "A" else 1024], F32,
                           name=f"ps{pool_key}", tag=f"ps{pool_key}")
            for b, off in blocks:
                q0, q1 = _block_region(b)
                kblk = kg[:, poff + 128 * b: poff + 128 * (b + 1)]
                # split matmuls at psum bank boundaries (512 fp32 cols),
                # in tile coords: block cols live at [off, off + q1-q0)
                a = q0
                while a < q1:
                    ta = a - q0 + off
                    bank_end = ((ta // 512) + 1) * 512
                    e = min(q1, a + (bank_end - ta))
                    nc.tensor.matmul(
                        ps[:, ta:ta + (e - a)],
                        lhsT=kblk,
                        rhs=qg[:, poff + a: poff + e],
                        start=True, stop=True,
                    )
                    a = e
            pt = ptp.tile([128, span], F16, name=f"pt{tag}", tag=f"pt{tag}")
            nc.scalar.activation(
                out=pt, in_=ps[:, 0:span], func=AF.Exp, scale=SCALE,
            )
            state[("pt", p, ui)] = pt

        def emit_masks(p, uis):
            """diagonal masks for the blocks of units uis (DVE, in-place)."""
            for ui in uis:
                _, _, blocks, _ = UNITS[ui]
                pt = state[("pt", p, ui)]
                for b, off in blocks:
                    nc.vector.tensor_mul(
                        pt[:, off:off + 128], pt[:, off:off + 128], tri_t
                    )

        def emit_pv(p):
            """transposed PV with fused denominator: for each q-block qb,
            out[q, 0:129] = sum_{kb<=qb} P_kb[:, qb].T @ [V_kb | 1]."""
            g, pi = divmod(p, NCHUNK)
            qg, kg, vg = state[("grp", g)]
            voff = pi * PW
            pso = psOp.tile([128, 1536], F32, tag="psO")
            for qb in range(NB):
                o0 = QB_OFF[qb]
                for kb in range(qb + 1):
                    ui, off = BLK[kb]
                    pt = state[("pt", p, ui)]
                    # lhsT: P columns for q-block qb within block kb
                    c0 = off + 128 * (qb - kb)
                    nc.tensor.matmul(
                        pso[:, o0:o0 + VW],
                        lhsT=pt[:, c0:c0 + 128],
                        rhs=vg[:, voff + VS * kb: voff + VS * kb + VW],
                        start=(kb == 0), stop=(kb == qb),
                    )
            for ui in range(4):
                state.pop(("pt", p, ui))
            state[("psO", p)] = pso

        def emit_drains(p):
            """compact psO -> outg (fp16) and DMA per finished group."""
            g, pi = divmod(p, NCHUNK)
            pso = state.pop(("psO", p))
            if pi == 0:
                state[("outg", g)] = outp.tile(
                    [128, GO], F16, name="outg", tag="outg"
                )
            outg = state[("outg", g)]
            o = pi * POW
            for s0, s1, d0 in DRAINS:
                nc.vector.tensor_copy(
                    out=outg[:, o + d0: o + d0 + (s1 - s0)], in_=pso[:, s0:s1]
                )
            if pi == NCHUNK - 1:
                nc.sync.dma_start(
                    out=outT.ap()[:, g * GO:(g + 1) * GO], in_=outg
                )
                state.pop(("outg", g))
                state.pop(("grp", g))

        # software-pipelined main loop. Per step p (engine queue order):
        #   PE : scores uA(p), uB(p) | PV(p-1) | scores uC(p), uD(p)
        #   ACT: exp uA(p), uB(p), uC(p), uD(p)
        #   DVE: drains(p-1), masks(p)
        for p in range(nprob + 1):
            if p < nprob:
                if p == 0:
                    emit_loads(0)
                if p % NCHUNK == 1 and (p // NCHUNK) + 1 < ngroups:
                    emit_loads(p // NCHUNK + 1)
                emit_scores_exp_unit(p, 0)
                emit_scores_exp_unit(p, 1)
            if p > 0:
                emit_pv(p - 1)
                emit_drains(p - 1)
            if p < nprob:
                emit_masks(p, [0, 1])
                emit_scores_exp_unit(p, 2)
                emit_scores_exp_unit(p, 3)
                emit_masks(p, [2, 3])


def _host_consts():
    freqs = np.exp(np.arange(HALF, dtype=np.float64) * (-math.log(ROPE_BASE) / HALF))
    pos = np.arange(L, dtype=np.float64)
    ang = pos[:, None] * freqs[None, :]  # (L, 64)
    cos = np.cos(ang)
    sin = np.sin(ang)
    r = np.arange(128)
    tri = (r[None, :] >= r[:, None]).astype(np.float16)  # keep q >= k
    return cos, sin, tri


def _rope_host(x, cos, sin):
    """x: (B, L, HPC, 128) fp32; cos/sin: (L, 64)."""
    x1, x2 = x[..., :HALF], x[..., HALF:]
    c = cos[None, :, None, :]
    s = sin[None, :, None, :]
    return np.concatenate([x1 * c - x2 * s, x2 * c + x1 * s], axis=-1)


def _pack_core(qc, kc, vc, cos, sin):
    """qc,kc,vc: (B, L, HPC, 128) fp32 -> device input maps."""
    qc = _rope_host(qc, cos, sin)
    kc = _rope_host(kc, cos, sin)

    def dmaj(x):
        # (B, L, h, D) -> (b, h, n, j, d) -> (d, b, h, n, j)
        a = x.transpose(0, 2, 1, 3).reshape(B, HPC, NCHUNK, CHUNK, DH)
        a = a.transpose(4, 0, 1, 2, 3).reshape(DH, NPROB * CHUNK)
        return np.ascontiguousarray(a).astype(np.float16)

    # v: partition = k-within-block, cols = (b,h,n, block, dv|1|pad3)
    a = vc.transpose(0, 2, 1, 3).reshape(B, HPC, NCHUNK, NB, 128, DV)
    ext = np.zeros(a.shape[:-1] + (VS,), a.dtype)
    ext[..., :DV] = a
    ext[..., DV] = 1.0
    ext = ext.transpose(4, 0, 1, 2, 3, 5).reshape(128, NPROB * PW)
    vp = np.ascontiguousarray(ext).astype(np.float16)
    return dict(qT_in=dmaj(qc), kT_in=dmaj(kc), vT_in=vp)


_NC_CACHE = {}
LAST_RESULT = None


def _get_module(nprob=NPROB):
    if nprob not in _NC_CACHE:
        _NC_CACHE[nprob] = build_module(nprob)
    return _NC_CACHE[nprob]


def kernel(q, k, v):
    q = np.asarray(q, dtype=np.float32)
    k = np.asarray(k, dtype=np.float32)
    v = np.asarray(v, dtype=np.float32)

    cos, sin, tri = _host_consts()
    consts = dict(tri_in=tri)

    in_maps = []
    for c in range(NCORES):
        hs = slice(HPC * c, HPC * (c + 1))
        m = _pack_core(q[:, :, hs], k[:, :, hs], v[:, :, hs], cos, sin)
        m.update(consts)
        in_maps.append(m)

    nc = _get_module(NPROB)
    trace = bool(int(os.environ.get("KERNEL_TRACE", "0")))
    res = bass_utils.run_bass_kernel_spmd(
        nc, in_maps, core_ids=list(range(NCORES)), trace=trace
    )
    global LAST_RESULT
    LAST_RESULT = res

    out = np.empty((B, L, H, DV), np.float32)
    for c in range(NCORES):
        ot = res.results[c]["outT_out"].astype(np.float32)  # (128q, 32*1040)
        # cols: (p, qb, d) with d in [0,130); partitions: q within q-block
        o = ot.reshape(128, NPROB, NB, OW)
        num = o[..., :DV]  # (q, p, qb, dv)
        den = o[..., DV]  # (q, p, qb)
        r = num / den[..., None]
        # (q, p, qb, dv) -> (p, qb, q, dv) -> (b, h, n, l_in_chunk, dv)
        r = r.transpose(1, 2, 0, 3).reshape(B, HPC, NCHUNK, CHUNK, DV)
        # -> (b, n, j, h, dv) -> (b, L, h, dv)
        r = r.transpose(0, 2, 3, 1, 4).reshape(B, L, HPC, DV)
        out[:, :, HPC * c:HPC * (c + 1)] = r
    return out


# revision 15
# speedup vs baseline: 1.2824x; 1.0594x over previous
"""Chunked causal self-attention with RoPE on 8 Trainium2 NeuronCores.

Problem: B=4, L=4096, H=16, DH=DV=128, CHUNK=1024 (N=4 chunks).
RoPE on q,k then chunk-local causal attention per (batch, chunk, head).

Sharding: heads split across 8 cores (2 heads/core) -> 32 independent
(1024 x 1024, d=128) attention problems per core, grouped 4-per-(b,h)
so one load covers a whole group.

v3 design notes (from trace analysis of v2: DVE was 89.5% busy on the
softmax-denominator R-build + fp32 PSUM drains; ACT 69%, PE 72%):
  - Fused denominator: PV runs transposed, out[q,dv] = sum_k P[k,q] *
    Vext[k,dv] with Vext = [V | ones]; the 129th output column IS the
    softmax denominator. This deletes the DVE R-build, the PE den
    matmuls and the separate den drain.
  - exp in 4 activations/problem (block units {b0,b4} {b1,b7} {b2,b3}
    {b5,b6} = 1536/1024/1408/640 cols) to amortize ScalarE overhead.
  - PSUM: psA [128,1536] (3 banks) + psB [128,1024] (2) ping-pong for
    scores; psO [128,1536] (3) holds the 8 q-block PV outputs at
    offsets {0,129,258, 512,641,770, 1024,1153} so no matmul output
    crosses a bank. Total exactly 8 banks.
  - DVE only does 8 diag masks + 3 compacting PSUM->SBUF casts per
    problem (~3us vs ~7us in v2).
  - Normalization (num/den) and final layout transposes on host.
"""

import math
import os
import sys

import numpy as np

for _p in ("/opt/trn_rl_repo", "/root/.axon_site/_ro/trn_rl_repo"):
    if os.path.isdir(_p) and _p not in sys.path:
        sys.path.insert(0, _p)

import concourse.bass as bass  # noqa: E402
import concourse.tile as tile  # noqa: E402
from concourse import bass_utils, mybir  # noqa: E402

B, L, H, DH, DV = 4, 4096, 16, 128, 128
CHUNK = 1024
NCHUNK = L // CHUNK  # 4
ROPE_BASE = 10000.0
NCORES = 8
HPC = H // NCORES  # 2 heads per core
NPROB = B * HPC * NCHUNK  # 32 problems per core
NG = B * HPC  # 8 groups of 4 chunks
HALF = DH // 2  # 64
NB = CHUNK // 128  # 8 k-blocks
SCALE = 1.0 / math.sqrt(DH)
VW = 130  # v block width fed to PV: 128 dv + ones col (den) + zero pad
VS = 132  # v block stride in SBUF/HBM (8B-aligned: 132*2B = 264B)
PW = NB * VS  # 1056 packed v cols per problem
OW = 130  # psO q-block region stride (130*4B = 520B, 8B-aligned)
POW = NB * OW  # 1040 packed output cols per problem

F16 = mybir.dt.float16
F32 = mybir.dt.float32
AF = mybir.ActivationFunctionType

# exp units: (tag, psum pool key, [(block, tile_col_off)], span)
# unit tile sizes: A=1536 (3 banks), B=1024 (2 banks); A and B ping-pong.
UNITS = [
    ("uA", "A", [(0, 0), (4, 1024)], 1536),
    ("uB", "B", [(1, 0), (7, 896)], 1024),
    ("uC", "A", [(2, 0), (3, 768)], 1408),
    ("uD", "B", [(5, 0), (6, 384)], 640),
]
# block -> (unit index, tile col offset of block start)
BLK = {}
for _ui, (_, _, _blocks, _) in enumerate(UNITS):
    for _b, _off in _blocks:
        BLK[_b] = (_ui, _off)

# psO column offset of each q-block's 129-wide output region (bank safe:
# banks hold 512 fp32; 3+3+2 regions per bank; 8B-aligned starts).
QB_OFF = [0, 130, 260, 512, 642, 772, 1024, 1154]
# drain: (psO src range) -> (outg dst offset); keeps the 130-col stride
DRAINS = [(0, 390, 0), (512, 902, 390), (1024, 1284, 780)]


def _block_region(b):
    """absolute q range covered for k-block b (causal)."""
    return 128 * b, CHUNK


def build_module(nprob=NPROB):
    from concourse import bacc

    nc = bacc.Bacc("TRN2", target_bir_lowering=False, debug=False)

    qT = nc.dram_tensor("qT_in", (128, nprob * CHUNK), F16, kind="ExternalInput")
    kT = nc.dram_tensor("kT_in", (128, nprob * CHUNK), F16, kind="ExternalInput")
    vT = nc.dram_tensor("vT_in", (128, nprob * PW), F16, kind="ExternalInput")
    tri = nc.dram_tensor("tri_in", (128, 128), F16, kind="ExternalInput")

    outT = nc.dram_tensor("outT_out", (128, nprob * POW), F16, kind="ExternalOutput")

    with tile.TileContext(nc) as tc:
        _body(tc, nprob, qT, kT, vT, tri, outT)
    nc.compile()
    return nc


def _body(tc, nprob, qT, kT, vT, tri, outT):
    from contextlib import ExitStack

    nc = tc.nc
    ngroups = nprob // NCHUNK
    GW = NCHUNK * CHUNK  # q/k group width: 4096 cols
    GV = NCHUNK * PW  # v group width: 4224 cols
    GO = NCHUNK * POW  # out group width: 4160 cols

    with ExitStack() as ctx:
        consts = ctx.enter_context(tc.tile_pool(name="consts", bufs=1))
        ing = ctx.enter_context(tc.tile_pool(name="ing", bufs=2))
        ptp = ctx.enter_context(tc.tile_pool(name="ptp", bufs=2))
        outp = ctx.enter_context(tc.tile_pool(name="outp", bufs=2))
        psAp = ctx.enter_context(tc.tile_pool(name="psA", bufs=1, space="PSUM"))
        psBp = ctx.enter_context(tc.tile_pool(name="psB", bufs=1, space="PSUM"))
        psOp = ctx.enter_context(tc.tile_pool(name="psO", bufs=1, space="PSUM"))

        tri_t = consts.tile([128, 128], F16, tag="tri")
        nc.sync.dma_start(out=tri_t, in_=tri.ap())

        # touch consts once so compute ops don't carry extra DMA waits
        dummy = consts.tile([128, 1], F16, tag="dummy")
        nc.vector.tensor_copy(out=dummy, in_=tri_t[:, 0:1])

        state = {}

        def emit_loads(g):
            # split DMAs so the first problem's slice lands early
            # (subtile deps let its scores start before the rest arrives)
            qg = ing.tile([128, GW], F16, tag="qg")
            nc.sync.dma_start(out=qg[:, 0:CHUNK],
                              in_=qT.ap()[:, g * GW:g * GW + CHUNK])
            nc.sync.dma_start(out=qg[:, CHUNK:GW],
                              in_=qT.ap()[:, g * GW + CHUNK:(g + 1) * GW])
            kg = ing.tile([128, GW], F16, tag="kg")
            nc.sync.dma_start(out=kg[:, 0:CHUNK],
                              in_=kT.ap()[:, g * GW:g * GW + CHUNK])
            nc.sync.dma_start(out=kg[:, CHUNK:GW],
                              in_=kT.ap()[:, g * GW + CHUNK:(g + 1) * GW])
            vg = ing.tile([128, GV], F16, tag="vg")
            nc.sync.dma_start(out=vg, in_=vT.ap()[:, g * GV:(g + 1) * GV])
            state[("grp", g)] = (qg, kg, vg)

        def emit_scores_exp_unit(p, ui):
            """score matmuls + one exp for unit ui of problem p."""
            g, pi = divmod(p, NCHUNK)
            qg, kg, vg = state[("grp", g)]
            poff = pi * CHUNK
            tag, pool_key, blocks, span = UNITS[ui]
            pool = psAp if pool_key == "A" else psBp
            ps = pool.tile([128, 1536 if pool_key == "A" else 1024], F32,
                           name=f"ps{pool_key}", tag=f"ps{pool_key}")
            for b, off in blocks:
                q0, q1 = _block_region(b)
                kblk = kg[:, poff + 128 * b: poff + 128 * (b + 1)]
                # split matmuls at psum bank boundaries (512 fp32 cols),
                # in tile coords: block cols live at [off, off + q1-q0)
                a = q0
                while a < q1:
                    ta = a - q0 + off
                    bank_end = ((ta // 512) + 1) * 512
                    e = min(q1, a + (bank_end - ta))
                    nc.tensor.matmul(
                        ps[:, ta:ta + (e - a)],
                        lhsT=kblk,
                        rhs=qg[:, poff + a: poff + e],
                        start=True, stop=True,
                    )
                    a = e
            pt = ptp.tile([128, span], F16, name=f"pt{tag}", tag=f"pt{tag}")
            nc.scalar.activation(
                out=pt, in_=ps[:, 0:span], func=AF.Exp, scale=SCALE,
            )
            state[("pt", p, ui)] = pt

        def emit_masks(p, uis):
            """diagonal masks for the blocks of units uis (DVE, in-place)."""
            for ui in uis:
                _, _, blocks, _ = UNITS[ui]
                pt = state[("pt", p, ui)]
                for b, off in blocks:
                    nc.vector.tensor_mul(
                        pt[:, off:off + 128], pt[:, off:off + 128], tri_t
                    )

        def emit_pv(p, qbs):
            """transposed PV with fused denominator: for each q-block qb,
            out[q, 0:130] = sum_{kb<=qb} P_kb[:, qb].T @ [V_kb | 1 | 0]."""
            g, pi = divmod(p, NCHUNK)
            qg, kg, vg = state[("grp", g)]
            voff = pi * PW
            if qbs[0] == 0:
                state[("psO", p)] = psOp.tile(
                    [128, 1536], F32, name="pso", tag="psO"
                )
            pso = state[("psO", p)]
            for qb in qbs:
                o0 = QB_OFF[qb]
                for kb in range(qb + 1):
                    ui, off = BLK[kb]
                    pt = state[("pt", p, ui)]
                    # lhsT: P columns for q-block qb within block kb
                    c0 = off + 128 * (qb - kb)
                    nc.tensor.matmul(
                        pso[:, o0:o0 + VW],
                        lhsT=pt[:, c0:c0 + 128],
                        rhs=vg[:, voff + VS * kb: voff + VS * kb + VW],
                        start=(kb == 0), stop=(kb == qb),
                    )
            if qbs[-1] == NB - 1:
                for ui in range(4):
                    state.pop(("pt", p, ui))
                if pi == NCHUNK - 1:
                    state.pop(("grp", g))

        def emit_drains(p, parts):
            """compact psO -> per-problem out tile (fp16), DMA when done."""
            if parts[0] == 0:
                state[("outp", p)] = outp.tile(
                    [128, POW], F16, name="outt", tag="outt"
                )
            outg = state[("outp", p)]
            pso = state[("psO", p)]
            for i in parts:
                s0, s1, d0 = DRAINS[i]
                nc.vector.tensor_copy(
                    out=outg[:, d0: d0 + (s1 - s0)], in_=pso[:, s0:s1]
                )
            if parts[-1] == len(DRAINS) - 1:
                nc.sync.dma_start(
                    out=outT.ap()[:, p * POW:(p + 1) * POW], in_=outg
                )
                state.pop(("outp", p))
                state.pop(("psO", p))

        # software-pipelined main loop. Per step p (engine queue order):
        #   PE : scores uA(p), uB(p) | PV(p-1) qb0-5 | scores uC(p) |
        #        PV(p-1) qb6-7 | scores uD(p)
        #   ACT: exp uA(p), uB(p), uC(p), uD(p)
        #   DVE: drains(p-1), masks(p)
        for p in range(nprob + 1):
            if p < nprob:
                if p == 0:
                    emit_loads(0)
                if p % NCHUNK == 1 and (p // NCHUNK) + 1 < ngroups:
                    emit_loads(p // NCHUNK + 1)
                emit_scores_exp_unit(p, 0)
                emit_scores_exp_unit(p, 1)
            if p > 0:
                emit_pv(p - 1, [0, 1, 2, 3, 4, 5])
                emit_drains(p - 1, [0, 1])
            if p < nprob:
                emit_masks(p, [0, 1])
                emit_scores_exp_unit(p, 2)
            if p > 0:
                emit_pv(p - 1, [6, 7])
                emit_drains(p - 1, [2])
            if p < nprob:
                emit_scores_exp_unit(p, 3)
                emit_masks(p, [2, 3])


def _host_consts():
    freqs = np.exp(np.arange(HALF, dtype=np.float64) * (-math.log(ROPE_BASE) / HALF))
    pos = np.arange(L, dtype=np.float64)
    ang = pos[:, None] * freqs[None, :]  # (L, 64)
    cos = np.cos(ang)
    sin = np.sin(ang)
    r = np.arange(128)
    tri = (r[None, :] >= r[:, None]).astype(np.float16)  # keep q >= k
    return cos, sin, tri


def _rope_host(x, cos, sin):
    """x: (B, L, HPC, 128) fp32; cos/sin: (L, 64)."""
    x1, x2 = x[..., :HALF], x[..., HALF:]
    c = cos[None, :, None, :]
    s = sin[None, :, None, :]
    return np.concatenate([x1 * c - x2 * s, x2 * c + x1 * s], axis=-1)


def _pack_core(qc, kc, vc, cos, sin):
    """qc,kc,vc: (B, L, HPC, 128) fp32 -> device input maps."""
    qc = _rope_host(qc, cos, sin)
    kc = _rope_host(kc, cos, sin)

    def dmaj(x):
        # (B, L, h, D) -> (b, h, n, j, d) -> (d, b, h, n, j)
        a = x.transpose(0, 2, 1, 3).reshape(B, HPC, NCHUNK, CHUNK, DH)
        a = a.transpose(4, 0, 1, 2, 3).reshape(DH, NPROB * CHUNK)
        return np.ascontiguousarray(a).astype(np.float16)

    # v: partition = k-within-block, cols = (b,h,n, block, dv|1|pad3)
    a = vc.transpose(0, 2, 1, 3).reshape(B, HPC, NCHUNK, NB, 128, DV)
    ext = np.zeros(a.shape[:-1] + (VS,), a.dtype)
    ext[..., :DV] = a
    ext[..., DV] = 1.0
    ext = ext.transpose(4, 0, 1, 2, 3, 5).reshape(128, NPROB * PW)
    vp = np.ascontiguousarray(ext).astype(np.float16)
    return dict(qT_in=dmaj(qc), kT_in=dmaj(kc), vT_in=vp)


_NC_CACHE = {}
LAST_RESULT = None


def _get_module(nprob=NPROB):
    if nprob not in _NC_CACHE:
        _NC_CACHE[nprob] = build_module(nprob)
    return _NC_CACHE[nprob]


def kernel(q, k, v):
    q = np.asarray(q, dtype=np.float32)
    k = np.asarray(k, dtype=np.float32)
    v = np.asarray(v, dtype=np.float32)

    cos, sin, tri = _host_consts()
    consts = dict(tri_in=tri)

    in_maps = []
    for c in range(NCORES):
        hs = slice(HPC * c, HPC * (c + 1))
        m = _pack_core(q[:, :, hs], k[:, :, hs], v[:, :, hs], cos, sin)
        m.update(consts)
        in_maps.append(m)

    nc = _get_module(NPROB)
    trace = bool(int(os.environ.get("KERNEL_TRACE", "0")))
    res = bass_utils.run_bass_kernel_spmd(
        nc, in_maps, core_ids=list(range(NCORES)), trace=trace
    )
    global LAST_RESULT
    LAST_RESULT = res

    out = np.empty((B, L, H, DV), np.float32)
    for c in range(NCORES):
        ot = res.results[c]["outT_out"].astype(np.float32)  # (128q, 32*1040)
        # cols: (p, qb, d) with d in [0,130); partitions: q within q-block
        o = ot.reshape(128, NPROB, NB, OW)
        num = o[..., :DV]  # (q, p, qb, dv)
        den = o[..., DV]  # (q, p, qb)
        r = num / den[..., None]
        # (q, p, qb, dv) -> (p, qb, q, dv) -> (b, h, n, l_in_chunk, dv)
        r = r.transpose(1, 2, 0, 3).reshape(B, HPC, NCHUNK, CHUNK, DV)
        # -> (b, n, j, h, dv) -> (b, L, h, dv)
        r = r.transpose(0, 2, 3, 1, 4).reshape(B, L, HPC, DV)
        out[:, :, HPC * c:HPC * (c + 1)] = r
    return out
